# revision 1
# baseline (speedup 1.0000x reference)
"""GQA attention (B=2, S=2048, 16 q heads / 4 kv heads, head_dim=128) with RoPE
and causal softmax, tensor-parallel over heads x data-parallel over batch on
8 Trainium2 NeuronCores.

Core c (0..7): batch b = c//4, TP rank r = c%4.
Each core computes, for its batch and its 4 q heads / 1 kv head:
  QT/KT/VT projections (transposed layout, contraction on partitions),
  RoPE via a signed pair-swap permutation matmul + elementwise muls,
  causal softmax without max-subtraction (scores bounded; exp in fp32->bf16),
  P@V in transposed layout (no transposes of the probability tiles),
  row-sharded output projection producing a partial [DIM, S] f32 output.
Host sums the 4 partials per batch (the row-parallel all-reduce) + transposes.
"""

import numpy as np
import ml_dtypes
from contextlib import ExitStack

import concourse.bass as bass
import concourse.tile as tile
from concourse import bacc, mybir, bass_utils, masks

B, S, DIM = 2, 2048, 2048
NH, NKV, HD = 16, 4, 128
TPR = 4            # tensor-parallel ranks per batch
LQH = NH // TPR    # 4 local q heads
QB = 512           # q block (free dim of matmuls)
NQB = S // QB      # 4
NDT = DIM // 128   # 16 contraction tiles for the projections
NKT = S // 128     # 16 key tiles
SCALE = 1.0 / float(np.sqrt(HD))

BF = mybir.dt.bfloat16
F32 = mybir.dt.float32


def _build(reps=1, bench_outs=None):
    nc = bacc.Bacc("TRN2", target_bir_lowering=False, debug=False, num_devices=8)

    xt_d = nc.dram_tensor("xt", [DIM, S], BF, kind="ExternalInput").ap()
    wq_d = nc.dram_tensor("wq", [DIM, LQH * HD], BF, kind="ExternalInput").ap()
    wk_d = nc.dram_tensor("wk", [DIM, HD], BF, kind="ExternalInput").ap()
    wv_d = nc.dram_tensor("wv", [DIM, HD], BF, kind="ExternalInput").ap()
    wo_d = nc.dram_tensor("wo", [LQH * HD, DIM], BF, kind="ExternalInput").ap()
    ce_d = nc.dram_tensor("ce", [HD, S], BF, kind="ExternalInput").ap()
    se_d = nc.dram_tensor("se", [HD, S], BF, kind="ExternalInput").ap()
    psw_d = nc.dram_tensor("psw", [HD, HD], BF, kind="ExternalInput").ap()
    n_outs = bench_outs if bench_outs is not None else reps
    assert reps <= n_outs
    out_ds = [
        nc.dram_tensor("out" if r == 0 else f"out{r}", [DIM, S], BF,
                       kind="ExternalOutput").ap()
        for r in range(n_outs)
    ]

    with tile.TileContext(nc, trace_sim=False) as tc, ExitStack() as ctx:
        persist = ctx.enter_context(tc.tile_pool(name="persist", bufs=1))
        xt_pool = ctx.enter_context(tc.tile_pool(name="xtq", bufs=3))
        work = ctx.enter_context(tc.tile_pool(name="work", bufs=3))
        expp = ctx.enter_context(tc.tile_pool(name="expp", bufs=12))
        outp = ctx.enter_context(tc.tile_pool(name="outp", bufs=4))
        # PSUM budget (8 banks total): pacc 2 + pscr 3 + pmix 2 + ptr 1
        psum = ctx.enter_context(tc.tile_pool(name="psum", bufs=1, space="PSUM"))

        for _rep in range(reps):
          out_d = out_ds[_rep]
          x_src = xt_d if _rep == 0 else out_ds[_rep - 1]
          # ---- persistent SBUF tensors ----
          wq_sb = persist.tile([128, NDT * LQH * HD], BF, tag="wq")   # dt-major blocks of 512
          wk_sb = persist.tile([128, NDT * HD], BF, tag="wk")
          wv_sb = persist.tile([128, NDT * HD], BF, tag="wv")
          wo_sb = persist.tile([128, LQH * DIM], BF, tag="wo")        # h-major blocks of 2048
          ce_sb = persist.tile([128, S], BF, tag="ce")
          se_sb = persist.tile([128, S], BF, tag="se")
          psw_sb = persist.tile([128, 128], BF, tag="psw")
          ones_sb = persist.tile([128, 128], BF, tag="ones")
          ident_sb = persist.tile([128, 128], BF, tag="ident")

          qt_sb = persist.tile([128, LQH * S], BF, tag="qt")          # roped Q^T per head
          kt_sb = persist.tile([128, S], BF, tag="kt")                # roped K^T
          vt_sb = persist.tile([128, S], BF, tag="vt")                # V^T
          vn_sb = persist.tile([128, S], BF, tag="vn")                # V natural, 16 x [128,128]
          ot_sb = persist.tile([128, LQH * S], BF, tag="ot")          # normalized attn out^T

          def dma(out_ap, in_ap):
              nc.sync.dma_start(out_ap, in_ap)

          def dma_rows(sb_ap, dram_ap, groups, cols):
              # one DMA for `groups` consecutive 128-row blocks of a row-major
              # DRAM matrix into column-blocks of a [128, groups*cols] SBUF tile
              dma(
                  sb_ap.rearrange("p (t q) -> p t q", t=groups),
                  dram_ap.rearrange("(t p) q -> p t q", p=128),
              )

          xq0 = xt_pool.tile([128, NDT * QB], BF, tag="xq")
          dma_rows(xq0[:, 0:QB], x_src[0:128, 0:QB], 1, QB)
          dma_rows(wq_sb[:, 0:512], wq_d[0:128, :], 1, 512)
          dma_rows(xq0[:, QB:4 * QB], x_src[128:512, 0:QB], 3, QB)
          dma_rows(wq_sb[:, 512:4 * 512], wq_d[128:512, :], 3, 512)
          for g in range(1, 4):
              dma_rows(
                  xq0[:, g * 4 * QB:(g + 1) * 4 * QB],
                  x_src[g * 512:(g + 1) * 512, 0:QB], 4, QB,
              )
              dma_rows(
                  wq_sb[:, g * 4 * 512:(g + 1) * 4 * 512],
                  wq_d[g * 512:(g + 1) * 512, :], 4, 512,
              )
          dma_rows(wk_sb[:], wk_d[:], NDT, HD)
          dma_rows(wv_sb[:], wv_d[:], NDT, HD)
          dma(ce_sb[:], ce_d[:])
          dma(se_sb[:], se_d[:])
          dma(psw_sb[:], psw_d[:])
          nc.gpsimd.memset(ones_sb[:], 1.0)
          masks.make_identity(nc, ident_sb[:])

          def rope(psum_in, out_ap, qb):
              """out = in * C + (signed pair swap of in) * S, written as bf16."""
              pre = work.tile([128, QB], BF, tag="pre")
              nc.scalar.copy(pre[:], psum_in[:])
              pu = psum.tile([128, QB], F32, tag="pmix", bufs=1)
              nc.tensor.matmul(pu[:], psw_sb[:], pre[:], start=True, stop=True)
              t1 = work.tile([128, QB], F32, tag="t1")
              nc.vector.tensor_mul(t1[:], pre[:], ce_sb[:, qb * QB:(qb + 1) * QB])
              t2 = work.tile([128, QB], F32, tag="t2")
              nc.vector.tensor_mul(t2[:], pu[:], se_sb[:, qb * QB:(qb + 1) * QB])
              nc.vector.tensor_add(out_ap, t1[:], t2[:])

          # ---- per-qb pipeline: proj(qb) -> attn(qb) -> out-proj(qb) ----
          def proj_stage(qb, xq):
              for h in range(LQH):
                  pq = psum.tile([128, QB], F32, tag="pacc", bufs=2)
                  for dt in range(NDT):
                      nc.tensor.matmul(
                          pq[:],
                          wq_sb[:, dt * 512 + h * 128: dt * 512 + (h + 1) * 128],
                          xq[:, dt * QB:(dt + 1) * QB],
                          start=(dt == 0),
                          stop=(dt == NDT - 1),
                      )
                  rope(pq, qt_sb[:, h * S + qb * QB: h * S + (qb + 1) * QB], qb)
              pk = psum.tile([128, QB], F32, tag="pacc", bufs=2)
              for dt in range(NDT):
                  nc.tensor.matmul(
                      pk[:],
                      wk_sb[:, dt * 128:(dt + 1) * 128],
                      xq[:, dt * QB:(dt + 1) * QB],
                      start=(dt == 0),
                      stop=(dt == NDT - 1),
                  )
              rope(pk, kt_sb[:, qb * QB:(qb + 1) * QB], qb)
              pv = psum.tile([128, QB], F32, tag="pacc", bufs=2)
              for dt in range(NDT):
                  nc.tensor.matmul(
                      pv[:],
                      wv_sb[:, dt * 128:(dt + 1) * 128],
                      xq[:, dt * QB:(dt + 1) * QB],
                      start=(dt == 0),
                      stop=(dt == NDT - 1),
                  )
              nc.scalar.copy(vt_sb[:, qb * QB:(qb + 1) * QB], pv[:])
              for kt in range(4 * qb, 4 * qb + 4):
                  ptr = psum.tile([128, 128], BF, tag="pmix", bufs=1)
                  nc.tensor.transpose(
                      ptr[:], vt_sb[:, kt * 128:(kt + 1) * 128], ident_sb[:]
                  )
                  nc.scalar.copy(vn_sb[:, kt * 128:(kt + 1) * 128], ptr[:])

          def attn_stage(qb):
              # work items: (kt, q_off, q_w, mask_i); fulls first, then the two
              # 256-wide diagonal halves (skips the fully-masked upper corner)
              items = [(kt, 0, QB, None) for kt in range(4 * qb)]
              items += [
                  (4 * qb, 0, 256, 0),
                  (4 * qb + 1, 0, 256, 1),
                  (4 * qb, 256, 256, None),
                  (4 * qb + 1, 256, 256, None),
                  (4 * qb + 2, 256, 256, 0),
                  (4 * qb + 3, 256, 256, 1),
              ]
              # per-256-half first/last touch -> matmul start/stop flags
              flags = []
              for idx, (kt, q_off, q_w, mi) in enumerate(items):
                  halves = range(q_off // 256, (q_off + q_w) // 256)
                  first = all(
                      not any(
                          o2 // 256 <= hh < (o2 + w2) // 256
                          for (_, o2, w2, _) in items[:idx]
                      )
                      for hh in halves
                  )
                  last = all(
                      not any(
                          o2 // 256 <= hh < (o2 + w2) // 256
                          for (_, o2, w2, _) in items[idx + 1:]
                      )
                      for hh in halves
                  )
                  flags.append((first, last))

              for h in range(LQH):
                  po = psum.tile([128, QB], F32, tag="pacc", bufs=2)
                  pden = psum.tile([128, QB], F32, tag="pden", bufs=2)
                  SKEW = 4  # PV/denom matmuls trail the score/exp stream

                  def pv_mm(idx, es):
                      kt, q_off, q_w, _ = items[idx]
                      first, last = flags[idx]
                      nc.tensor.matmul(
                          po[:, q_off:q_off + q_w],
                          vn_sb[:, kt * 128:(kt + 1) * 128],
                          es[:, :q_w],
                          start=first,
                          stop=last,
                      )
                      nc.tensor.matmul(
                          pden[:, q_off:q_off + q_w],
                          ones_sb[:],
                          es[:, :q_w],
                          start=first,
                          stop=last,
                      )

                  es_ring = {}
                  for idx, (kt, q_off, q_w, mi) in enumerate(items):
                      pscr = psum.tile([128, QB], F32, tag="pscr", bufs=3)
                      nc.tensor.matmul(
                          pscr[:, :q_w],
                          kt_sb[:, kt * 128:(kt + 1) * 128],
                          qt_sb[:, h * S + qb * QB + q_off:
                                h * S + qb * QB + q_off + q_w],
                          start=True,
                          stop=True,
                      )
                      es = expp.tile([128, QB], BF, tag="es")
                      nc.scalar.activation(
                          es[:, :q_w], pscr[:, :q_w],
                          mybir.ActivationFunctionType.Exp, scale=SCALE,
                      )
                      if mi is not None:  # diagonal: causal mask via GPSIMD iota
                          nc.gpsimd.affine_select(
                              out=es[:, :q_w],
                              in_=es[:, :q_w],
                              compare_op=mybir.AluOpType.is_ge,
                              fill=0.0,
                              base=-128 * mi,
                              channel_multiplier=-1,
                              pattern=[[1, q_w]],
                          )
                      es_ring[idx] = es
                      if idx >= SKEW:
                          pv_mm(idx - SKEW, es_ring.pop(idx - SKEW))
                  for idx in range(max(0, len(items) - SKEW), len(items)):
                      pv_mm(idx, es_ring.pop(idx))
                  rec = work.tile([128, QB], F32, tag="rec")
                  nc.vector.reciprocal(rec[:], pden[:])
                  nc.vector.tensor_mul(
                      ot_sb[:, h * S + qb * QB: h * S + (qb + 1) * QB], po[:], rec[:]
                  )

          _p3tags = [("pacc", 2), ("pscr", 3), ("pmix", 1), ("pden", 2)]

          def out_stage(qb):
              for grp in range(4):
                  obg = outp.tile([128, 4 * QB], BF, tag="obg")
                  for i in range(4):
                      dct = grp * 4 + i
                      _t, _b = _p3tags[dct % 4]
                      pw = psum.tile([128, QB], F32, tag=_t, bufs=_b)
                      for h in range(LQH):
                          nc.tensor.matmul(
                              pw[:],
                              wo_sb[:, h * DIM + dct * 128: h * DIM + (dct + 1) * 128],
                              ot_sb[:, h * S + qb * QB: h * S + (qb + 1) * QB],
                              start=(h == 0),
                              stop=(h == LQH - 1),
                          )
                      nc.vector.tensor_copy(obg[:, i * QB:(i + 1) * QB], pw[:])
                  dma(
                      out_d[grp * 512:(grp + 1) * 512,
                            qb * QB:(qb + 1) * QB].rearrange(
                          "(t p) q -> p t q", p=128),
                      obg.rearrange("p (t q) -> p t q", t=4),
                  )

          xqs = {0: xq0}
          for qb in range(NQB):
              proj_stage(qb, xqs.pop(qb))
              if qb + 1 < NQB:  # prefetch next x block before attention fills time
                  xq = xt_pool.tile([128, NDT * QB], BF, tag="xq")
                  for g in range(4):
                      dma_rows(
                          xq[:, g * 4 * QB:(g + 1) * 4 * QB],
                          x_src[g * 512:(g + 1) * 512, (qb + 1) * QB:(qb + 2) * QB],
                          4, QB,
                      )
                  xqs[qb + 1] = xq
              if qb == 0:  # wo lands during attention(0)
                  for h in range(LQH):
                      dma(
                          wo_sb[:, h * DIM:(h + 1) * DIM],
                          wo_d[h * 128:(h + 1) * 128, :],
                      )
              attn_stage(qb)
              if qb > 0:  # deferred by one: fills attention's exp-latency bubbles
                  out_stage(qb - 1)
          out_stage(NQB - 1)

    nc.compile()
    return nc


_NC_CACHE = []


def _get_nc():
    if not _NC_CACHE:
        _NC_CACHE.append(_build())
    return _NC_CACHE[0]


def _host_consts():
    bf = ml_dtypes.bfloat16
    psw = np.zeros((HD, HD), np.float32)
    p = np.arange(HD // 2)
    psw[2 * p + 1, 2 * p] = -1.0  # U[2p] = -in[2p+1]
    psw[2 * p, 2 * p + 1] = 1.0   # U[2p+1] = +in[2p]
    return psw.astype(bf)


def _make_in_maps(x, cos, sin, wq, wk, wv, wo):
    bf = ml_dtypes.bfloat16
    ce = np.repeat(np.ascontiguousarray(cos.T), 2, axis=0).astype(bf)  # [128, S]
    se = np.repeat(np.ascontiguousarray(sin.T), 2, axis=0).astype(bf)
    psw = _host_consts()
    xt = [np.ascontiguousarray(x[b].T).astype(bf) for b in range(B)]
    in_maps = []
    for c in range(8):
        b, r = divmod(c, TPR)
        in_maps.append(
            {
                "xt": xt[b],
                "wq": np.ascontiguousarray(wq[:, r * 512:(r + 1) * 512]).astype(bf),
                "wk": np.ascontiguousarray(wk[:, r * 128:(r + 1) * 128]).astype(bf),
                "wv": np.ascontiguousarray(wv[:, r * 128:(r + 1) * 128]).astype(bf),
                "wo": np.ascontiguousarray(wo[r * 512:(r + 1) * 512, :]).astype(bf),
                "ce": ce,
                "se": se,
                "psw": psw,
            }
        )
    return in_maps


def _assemble(results):
    full = np.empty((B, S, DIM), np.float32)
    for b in range(B):
        acc = results[TPR * b]["out"].astype(np.float32)
        for r in range(1, TPR):
            acc += results[TPR * b + r]["out"].astype(np.float32)
        full[b] = acc.T
    return full


def kernel(x, cos, sin, wq, wk, wv, wo):
    x = np.asarray(x, np.float32)
    cos = np.asarray(cos, np.float32)
    sin = np.asarray(sin, np.float32)
    wq = np.asarray(wq, np.float32)
    wk = np.asarray(wk, np.float32)
    wv = np.asarray(wv, np.float32)
    wo = np.asarray(wo, np.float32)

    nc = _get_nc()
    in_maps = _make_in_maps(x, cos, sin, wq, wk, wv, wo)
    res = bass_utils.run_bass_kernel_spmd(nc, in_maps, core_ids=list(range(8)))
    return _assemble(res.results)


def run_traced(inputs):
    """Timing/profiling helper for test.py (not used by the grader)."""
    nc = _get_nc()
    in_maps = _make_in_maps(
        np.asarray(inputs["x"], np.float32),
        np.asarray(inputs["cos"], np.float32),
        np.asarray(inputs["sin"], np.float32),
        np.asarray(inputs["wq"], np.float32),
        np.asarray(inputs["wk"], np.float32),
        np.asarray(inputs["wv"], np.float32),
        np.asarray(inputs["wo"], np.float32),
    )
    res = bass_utils.run_bass_kernel_spmd(
        nc, in_maps, core_ids=list(range(8)), trace=True
    )
    return res

